# revision 1
# baseline (speedup 1.0000x reference)
# NF5 block-quantized linear (AXSLinearV2) on 8 Trainium2 cores — v2.
#
# v2 restructure vs v1:
#   - W quantized first; qw AllGather early; wT o-quarter-0 resident during
#     x-quant so the matmul pipelines with quantization.
#   - x pair-AllGather at chunk (128-row) granularity: matmul row-tiles start
#     as soon as their chunk has been exchanged.
#   - r-outer matmul (lhsT = x-panel stationary, 512+512 LDWEIGHTS total),
#     y produced in natural [r, o] orientation.
#   - chunk-level Erf batch (one ACT table-set switch pair per 128-row chunk).
#   - engine balance: DVE / ACT / GPSIMD all carry quant passes.

import os
import sys

for _p in ("/opt/trn_rl_repo", "/root/.axon_site/_ro/trn_rl_repo"):
    if os.path.isdir(_p) and _p not in sys.path:
        sys.path.insert(0, _p)

import numpy as np

C_ = 2.1538746940614564        # ndtri(1 - 1/64)
A_ = 16.0                      # warp: t = A*e + B*|e|  (k-16 domain)
B_ = -0.516129032258065
CA = 0.010089541448848835      # boundary-correction coefficients
CB = -0.00019778768532179383
CC = 0.00028827144861916877
KAPPA = 0.036405892028398036   # D1*sqrt(2*pi)/C
INVC = 0.46427956220347655
HC2 = 2.3195880606018394       # C^2/2 (Halley term, for h1 = sN^2 * (z/C))
ERFS = 0.7071067811865475
QFRAC = 0.9369999999999976     # quantile(0.999) lerp fraction for n=64
MAGIC = 12582912.0             # 1.5*2^23 round-to-nearest trick

NCORES = 8
B_SZ, S_SZ, D_IN, D_OUT = 4, 2048, 4096, 4096
RTOT = B_SZ * S_SZ
XR = RTOT // NCORES            # 1024 x-rows quantized per core
WR = D_OUT // NCORES           # 512 w-rows quantized per core
NXCH = XR // 128               # 8 x chunks
NWCH = WR // 128               # 4 w chunks

_cache = {}


def _build_nc(repeat=1, phase="all"):
    import concourse.bass as bass
    import concourse.bacc as bacc
    import concourse.tile as tile
    from concourse import mybir

    f32 = mybir.dt.float32
    bf16 = mybir.dt.bfloat16
    u32 = mybir.dt.uint32
    Alu = mybir.AluOpType
    Act = mybir.ActivationFunctionType

    def bcast64(ap2d):
        return bass.AP(tensor=ap2d.tensor, offset=ap2d.offset,
                       ap=[ap2d.ap[0], ap2d.ap[1], [0, 64]])

    nc = bacc.Bacc("TRN2", target_bir_lowering=False, debug=False,
                   num_devices=NCORES)
    x_sh = nc.dram_tensor("x_sh", [XR, D_IN], f32, kind="ExternalInput")
    w_sh = nc.dram_tensor("w_sh", [WR, D_IN], f32, kind="ExternalInput")
    bias_h = nc.dram_tensor("bias_h", [1, 2048], f32, kind="ExternalInput")
    y_sh = nc.dram_tensor("y_sh", [2048, 2048], f32, kind="ExternalOutput")

    with tile.TileContext(nc) as tc:
     for _rep in range(repeat):
        with tc.tile_pool(name="dram", bufs=1, space="DRAM") as dram:
            qx_own = dram.tile([XR, D_IN], bf16)
            qx_full = dram.tile([NXCH, 256, D_IN], bf16)   # [chunk][pair-slot*128][i]
            qw_own = dram.tile([WR, D_IN], bf16)
            qw_half = dram.tile([4 * WR, D_IN], bf16)

            with (
                tc.tile_pool(name="mmw1", bufs=1) as mmw1,
                tc.tile_pool(name="mmxp", bufs=2) as mmxp,
                tc.tile_pool(name="mmy", bufs=2) as mmy,
                tc.tile_pool(name="psum1", bufs=2, space="PSUM") as pp1,
                tc.tile_pool(name="misc", bufs=1) as misc,
            ):
                ones = misc.tile([1, 128], bf16)
                brow32 = misc.tile([1, 2048], f32)
                brow = misc.tile([1, 2048], bf16)
                if phase in ("all", "mm"):
                    nc.vector.memset(ones[:], 1.0)
                    nc.sync.dma_start(brow32[:], bias_h[:, :])
                    nc.vector.tensor_copy(brow[:], brow32[:])

                # ------------- quantization (+ chunk AGs + pass-1 mm) -------
                with (
                    tc.tile_pool(name="xin", bufs=2) as xin_pool,
                    tc.tile_pool(name="zpool", bufs=2) as zpool,
                    tc.tile_pool(name="epool", bufs=1) as epool,
                    tc.tile_pool(name="aux", bufs=2) as aux_pool,
                    tc.tile_pool(name="scale", bufs=2) as sc_pool,
                    tc.tile_pool(name="scl", bufs=10) as scl_pool,
                    tc.tile_pool(name="work", bufs=2) as work,
                    tc.tile_pool(name="qout", bufs=2) as qout_pool,
                    tc.tile_pool(name="qconst", bufs=1) as qconst,
                ):
                    biasCA = qconst.tile([128, 1], f32)
                    if phase in ("all", "quant"):
                        nc.vector.memset(biasCA[:], CA)

                    def quant_chunk(src, dst, r0):
                        """Quantize src[r0:r0+128, :] -> dst rows (bf16)."""
                        zfull = zpool.tile([128, 64, 64], f32, tag="z")
                        scs = []
                        xhalves = []
                        for h in range(2):
                            xh = xin_pool.tile([128, 32, 64], f32, tag="x")
                            nc.sync.dma_start(
                                xh[:], src[r0:r0 + 128,
                                           2048 * h:2048 * (h + 1)].rearrange(
                                    "r (g e) -> r g e", e=64))
                            xhalves.append(xh)
                        for j in range(8):       # scales + z, 512 cols each
                            xv = xhalves[j // 4][:, 8 * (j % 4):8 * (j % 4) + 8, :]
                            ax = aux_pool.tile([128, 8, 64], f32, tag="ax")
                            nc.vector.tensor_scalar(
                                out=ax[:].bitcast(u32), in0=xv.bitcast(u32),
                                scalar1=0x7FFFFFFF, scalar2=None,
                                op0=Alu.bitwise_and)
                            m8 = sc_pool.tile([128, 8, 8], f32, tag="m8")
                            for b in range(8):
                                nc.vector.max(out=m8[:, b, :], in_=ax[:, b, :])
                            dd = sc_pool.tile([128, 8], f32, tag="dd")
                            nc.vector.tensor_tensor(
                                out=dd[:], in0=m8[:, :, 0], in1=m8[:, :, 1],
                                op=Alu.subtract)
                            sc = scl_pool.tile([128, 8], f32, tag="sc")
                            nc.vector.scalar_tensor_tensor(
                                out=sc[:], in0=dd[:], scalar=QFRAC,
                                in1=m8[:, :, 1], op0=Alu.mult, op1=Alu.add)
                            nc.vector.tensor_scalar(
                                out=sc[:], in0=sc[:], scalar1=1e-8,
                                scalar2=None, op0=Alu.max)
                            rcpC = sc_pool.tile([128, 8], f32, tag="rcpC")
                            nc.vector.reciprocal(out=rcpC[:], in_=sc[:])
                            nc.vector.tensor_scalar(
                                out=rcpC[:], in0=rcpC[:], scalar1=C_,
                                scalar2=None, op0=Alu.mult)
                            nc.gpsimd.tensor_tensor(
                                out=zfull[:, 8 * j:8 * j + 8, :], in0=xv,
                                in1=bcast64(rcpC[:]), op=Alu.mult)
                            scs.append(sc)
                        zf2 = zfull[:].rearrange("p g e -> p (g e)")
                        nc.vector.tensor_scalar(
                            out=zf2, in0=zf2, scalar1=C_, scalar2=-C_,
                            op0=Alu.min, op1=Alu.max)
                        efull = epool.tile([128, 4096], f32, tag="e")
                        nc.scalar.activation(out=efull[:], in_=zf2,
                                             func=Act.Erf, scale=ERFS)
                        for j in range(8):       # warp + snap + value, 512 cols
                            sl = slice(512 * j, 512 * (j + 1))
                            zj = zfull[:].rearrange("p g e -> p (g e)")[:, sl]
                            ej = efull[:, sl]
                            z2 = work.tile([128, 512], f32, tag="T3")
                            nc.scalar.activation(out=z2[:], in_=zj,
                                                 func=Act.Square)
                            ab = work.tile([128, 512], f32, tag="T4")
                            nc.scalar.activation(out=ab[:], in_=ej,
                                                 func=Act.Abs, scale=B_)
                            tt = work.tile([128, 512], f32, tag="T5")
                            nc.vector.scalar_tensor_tensor(
                                out=tt[:], in0=ej, scalar=A_, in1=ab[:],
                                op0=Alu.mult, op1=Alu.subtract)
                            E2 = work.tile([128, 512], f32, tag="T6")
                            nc.scalar.activation(out=E2[:], in_=z2[:],
                                                 func=Act.Exp, scale=0.5)
                            w1 = work.tile([128, 512], f32, tag="T7")
                            nc.scalar.activation(out=w1[:], in_=z2[:],
                                                 func=Act.Identity,
                                                 bias=biasCA[:], scale=CB)
                            E2z = work.tile([128, 512], f32, tag="T3")
                            nc.gpsimd.tensor_tensor(
                                out=E2z[:], in0=E2[:], in1=zj, op=Alu.mult)
                            w2 = work.tile([128, 512], f32, tag="T4")
                            nc.vector.tensor_tensor(
                                out=w2[:], in0=E2z[:], in1=w1[:], op=Alu.mult)
                            azcc = work.tile([128, 512], f32, tag="T7")
                            nc.scalar.activation(out=azcc[:], in_=E2z[:],
                                                 func=Act.Abs, scale=CC)
                            tc1 = work.tile([128, 512], f32, tag="T3")
                            nc.vector.scalar_tensor_tensor(
                                out=tc1[:], in0=w2[:], scalar=-1.0, in1=tt[:],
                                op0=Alu.mult, op1=Alu.add)
                            nc.vector.scalar_tensor_tensor(
                                out=tc1[:], in0=azcc[:], scalar=-1.0,
                                in1=tc1[:], op0=Alu.mult, op1=Alu.add)
                            kk = work.tile([128, 512], f32, tag="T7")
                            nc.vector.tensor_scalar(
                                out=kk[:], in0=tc1[:], scalar1=MAGIC,
                                scalar2=MAGIC, op0=Alu.add, op1=Alu.subtract)
                            nc.vector.tensor_scalar(
                                out=kk[:], in0=kk[:], scalar1=-16.0,
                                scalar2=15.0, op0=Alu.max, op1=Alu.min)
                            du = work.tile([128, 512], bf16, tag="T8")
                            nc.gpsimd.tensor_tensor(
                                out=du[:], in0=tt[:], in1=kk[:],
                                op=Alu.subtract)
                            sNb = work.tile([128, 512], bf16, tag="T9")
                            nc.vector.scalar_tensor_tensor(
                                out=sNb[:], in0=du[:], scalar=KAPPA,
                                in1=E2[:], op0=Alu.mult, op1=Alu.mult)
                            zb = work.tile([128, 512], bf16, tag="T10")
                            nc.scalar.activation(out=zb[:], in_=zj,
                                                 func=Act.Copy, scale=INVC)
                            v = work.tile([128, 512], bf16, tag="T11")
                            nc.vector.tensor_tensor(
                                out=v[:], in0=zb[:], in1=sNb[:],
                                op=Alu.subtract)
                            sN2 = work.tile([128, 512], bf16, tag="T7")
                            nc.scalar.activation(out=sN2[:], in_=sNb[:],
                                                 func=Act.Square)
                            h1 = work.tile([128, 512], bf16, tag="T3")
                            nc.gpsimd.tensor_tensor(
                                out=h1[:], in0=sN2[:], in1=zb[:], op=Alu.mult)
                            nc.vector.scalar_tensor_tensor(
                                out=v[:], in0=h1[:], scalar=HC2, in1=v[:],
                                op0=Alu.mult, op1=Alu.add)
                            dq = qout_pool.tile([128, 8, 64], bf16, tag="dq")
                            nc.gpsimd.tensor_tensor(
                                out=dq[:],
                                in0=v[:].rearrange("p (g e) -> p g e", e=64),
                                in1=bcast64(scs[j][:]), op=Alu.mult)
                            nc.sync.dma_start(
                                dst[r0:r0 + 128, sl].rearrange(
                                    "r (g e) -> r g e", e=64), dq[:])

                    def mm_rt(ch, s, wt, o0, width, src):
                        """One row-tile of the matmul against resident wt."""
                        xp = mmxp.tile([128, 32, 128], bf16, tag="xp")
                        for k in range(32):
                            nc.sync.dma_start_transpose(
                                xp[:, k, :],
                                src[s * 128:(s + 1) * 128,
                                    k * 128:(k + 1) * 128])
                        ps = pp1.tile([128, width], f32, tag="ps")
                        for k in range(32):
                            for oc in range(width // 512):
                                nc.tensor.matmul(
                                    ps[:, oc * 512:(oc + 1) * 512],
                                    lhsT=xp[:, k, :],
                                    rhs=wt[:, k, oc * 512:(oc + 1) * 512],
                                    start=(k == 0), stop=False)
                        for oc in range(width // 512):
                            nc.tensor.matmul(
                                ps[:, oc * 512:(oc + 1) * 512], lhsT=ones[:],
                                rhs=brow[:, o0 + oc * 512:o0 + (oc + 1) * 512],
                                start=False, stop=True)
                        yb = mmy.tile([128, width], f32, tag="yb")
                        nc.scalar.copy(yb[:], ps[:])
                        rbase = s * 1024 + ch * 128
                        nc.sync.dma_start(
                            y_sh[rbase:rbase + 128, o0:o0 + width], yb[:])

                    # ---- W first, then its AG, then wT quarter-0 ----
                    if phase in ("all", "quant"):
                        for ch in range(NWCH):
                            quant_chunk(w_sh, qw_own, ch * 128)
                        nc.gpsimd.collective_compute(
                            "AllGather", Alu.bypass,
                            replica_groups=[[0, 2, 4, 6], [1, 3, 5, 7]],
                            ins=[qw_own.opt()], outs=[qw_half.opt()])
                    wt0 = mmw1.tile([128, 32, 1024], bf16, tag="wt0")
                    if phase in ("all", "mm"):
                        for k in range(32):
                            for oc in range(8):
                                nc.sync.dma_start_transpose(
                                    wt0[:, k, oc * 128:(oc + 1) * 128],
                                    qw_half[oc * 128:(oc + 1) * 128,
                                            k * 128:(k + 1) * 128])

                    # ---- X chunks: quant -> chunk AG -> pass-1 row tiles ----
                    for ch in range(NXCH):
                        if phase in ("all", "quant"):
                            quant_chunk(x_sh, qx_own, ch * 128)
                            nc.gpsimd.collective_compute(
                                "AllGather", Alu.bypass,
                                replica_groups=[[2 * i, 2 * i + 1]
                                                for i in range(4)],
                                ins=[qx_own[ch * 128:(ch + 1) * 128, :].opt()],
                                outs=[qx_full[ch].opt()])
                        if phase in ("all", "mm"):
                            for s in range(2):
                                mm_rt(ch, s, wt0, 0, 1024, qx_full[ch])

                # ---- passes 2-4: o in [512, 2048), bigger residency ----
                if phase in ("all", "mm"):
                    with (
                        tc.tile_pool(name="mmw2", bufs=1) as mmw2,
                        tc.tile_pool(name="psum2", bufs=2, space="PSUM") as pp2,
                    ):
                        wt3 = mmw2.tile([128, 32, 1024], bf16)
                        for k in range(32):
                            for oc in range(8):
                                nc.sync.dma_start_transpose(
                                    wt3[:, k, oc * 128:(oc + 1) * 128],
                                    qw_half[1024 + oc * 128:
                                            1024 + (oc + 1) * 128,
                                            k * 128:(k + 1) * 128])
                        for ch in range(NXCH):
                            for s in range(2):
                                xp = mmxp.tile([128, 32, 128], bf16, tag="xp")
                                for k in range(32):
                                    nc.sync.dma_start_transpose(
                                        xp[:, k, :],
                                        qx_full[ch, s * 128:(s + 1) * 128,
                                                k * 128:(k + 1) * 128])
                                ps = pp2.tile([128, 1024], f32, tag="ps2")
                                for k in range(32):
                                    for oc in range(2):
                                        nc.tensor.matmul(
                                            ps[:, oc * 512:(oc + 1) * 512],
                                            lhsT=xp[:, k, :],
                                            rhs=wt3[:, k,
                                                    oc * 512:(oc + 1) * 512],
                                            start=(k == 0), stop=False)
                                for oc in range(2):
                                    nc.tensor.matmul(
                                        ps[:, oc * 512:(oc + 1) * 512],
                                        lhsT=ones[:],
                                        rhs=brow[:, 1024 + oc * 512:
                                                 1024 + (oc + 1) * 512],
                                        start=False, stop=True)
                                yb = mmy.tile([128, 1024], f32, tag="yb")
                                nc.scalar.copy(yb[:], ps[:])
                                rbase = s * 1024 + ch * 128
                                nc.sync.dma_start(
                                    y_sh[rbase:rbase + 128, 1024:2048], yb[:])
    nc.compile()
    return nc


def kernel(input, weight, bias):
    from concourse.bass_utils import run_bass_kernel_spmd

    if "nc" not in _cache:
        _cache["nc"] = _build_nc(
            repeat=int(os.environ.get("KERNEL_REPEAT", "1")),
            phase=os.environ.get("KERNEL_PHASE", "all"))
    nc = _cache["nc"]

    x2 = np.ascontiguousarray(
        np.asarray(input, dtype=np.float32).reshape(RTOT, D_IN))
    w = np.asarray(weight, dtype=np.float32)
    b = np.asarray(bias, dtype=np.float32)

    in_maps = []
    for c in range(NCORES):
        ro, co = c // 2, c % 2
        xs = np.ascontiguousarray(x2[ro * 2048 + co * 1024:
                                     ro * 2048 + (co + 1) * 1024])
        ws = np.ascontiguousarray(w[co * 2048 + ro * 512:
                                    co * 2048 + (ro + 1) * 512])
        bh = np.ascontiguousarray(b[co * 2048:(co + 1) * 2048]).reshape(1, 2048)
        in_maps.append({"x_sh": xs, "w_sh": ws, "bias_h": bh})

    res = run_bass_kernel_spmd(nc, in_maps, core_ids=list(range(NCORES)))
    _cache["exec_time_ns"] = res.exec_time_ns

    y = np.empty((RTOT, D_OUT), dtype=np.float32)
    for c in range(NCORES):
        ro, co = c // 2, c % 2
        y[ro * 2048:(ro + 1) * 2048, co * 2048:(co + 1) * 2048] = \
            res.results[c]["y_sh"]
    return y.reshape(B_SZ, S_SZ, D_OUT)



# revision 2
# speedup vs baseline: 1.6260x; 1.6260x over previous
# NF5 block-quantized linear (AXSLinearV2) on 8 Trainium2 cores — v3.
#
# v3 restructure vs v2:
#   - 2x4 grid: row-groups of 4 cores share x (4-rank chunk AllGathers);
#     column pairs share quantized weights (two pair AllGathers, halves).
#   - wT resident in SBUF [128, 32k, 1024o] bf16 (8 MB), transposed-loaded
#     once via xbar DMA; single-pass mm (no second x sweep).
#   - pipeline: quant chunk j+1 (DVE/ACT/GPSIMD) overlaps mm chunk j (PE).
#     Emission order per iter: quant(j) -> AG(j) -> mm(j-1) so no engine
#     queue head-of-line blocking.
#   - loads on sync(SP) queue, stores (dq-out, y) on scalar(ACT) queue.

import os
import sys

for _p in ("/opt/trn_rl_repo", "/root/.axon_site/_ro/trn_rl_repo"):
    if os.path.isdir(_p) and _p not in sys.path:
        sys.path.insert(0, _p)

import numpy as np

C_ = 2.1538746940614564        # ndtri(1 - 1/64)
A_ = 16.0                      # warp: t = A*e + B*|e|  (k-16 domain)
B_ = -0.516129032258065
CA = 0.010089541448848835      # boundary-correction coefficients
CB = -0.00019778768532179383
CC = 0.00028827144861916877
KAPPA = 0.036405892028398036   # D1*sqrt(2*pi)/C
INVC = 0.46427956220347655
HC2 = 2.3195880606018394       # C^2/2 (Halley term, for h1 = sN^2 * (z/C))
ERFS = 0.7071067811865475
QFRAC = 0.9369999999999976     # quantile(0.999) lerp fraction for n=64
MAGIC = 12582912.0             # 1.5*2^23 round-to-nearest trick

NCORES = 8
B_SZ, S_SZ, D_IN, D_OUT = 4, 2048, 4096, 4096
RTOT = B_SZ * S_SZ
XR = RTOT // NCORES            # 1024 x-rows quantized per core
WR = D_OUT // NCORES           # 512 w-rows quantized per core
NXCH = XR // 128               # 8 x chunks
NWCH = WR // 128               # 4 w chunks
OCOLS = 1024                   # output cols per core (col-group of 4 cores)
YROWS = 4096                   # output rows per core (row-group rows)

_cache = {}


def _build_nc(repeat=1, phase="all"):
    import concourse.bass as bass
    import concourse.bacc as bacc
    import concourse.tile as tile
    from concourse import mybir

    f32 = mybir.dt.float32
    bf16 = mybir.dt.bfloat16
    u32 = mybir.dt.uint32
    Alu = mybir.AluOpType
    Act = mybir.ActivationFunctionType

    do_q = phase in ("all", "quant")
    do_m = phase in ("all", "mm")

    def bcast64(ap2d):
        return bass.AP(tensor=ap2d.tensor, offset=ap2d.offset,
                       ap=[ap2d.ap[0], ap2d.ap[1], [0, 64]])

    nc = bacc.Bacc("TRN2", target_bir_lowering=False, debug=False,
                   num_devices=NCORES)
    x_sh = nc.dram_tensor("x_sh", [XR, D_IN], f32, kind="ExternalInput")
    w_sh = nc.dram_tensor("w_sh", [WR, D_IN], f32, kind="ExternalInput")
    bias_h = nc.dram_tensor("bias_h", [1, OCOLS], f32, kind="ExternalInput")
    y_sh = nc.dram_tensor("y_sh", [YROWS, OCOLS], f32, kind="ExternalOutput")

    XG = [[0, 1, 2, 3], [4, 5, 6, 7]]          # row-groups (share x)
    WG = [[c, c + 4] for c in range(4)]        # col pairs (share w)

    with tile.TileContext(nc) as tc:
     for _rep in range(repeat):
        with tc.tile_pool(name="dram", bufs=1, space="DRAM") as dram:
            qx_own = dram.tile([XR, D_IN], bf16)
            qx_all = dram.tile([NXCH, 4, 128, D_IN], bf16)
            qw_own = dram.tile([WR, D_IN], bf16)
            qw_allA = dram.tile([2, 256, D_IN], bf16)   # w rows 0:256 of pair
            qw_allB = dram.tile([2, 256, D_IN], bf16)   # w rows 256:512

            with (
                tc.tile_pool(name="wts", bufs=1) as wts,
                tc.tile_pool(name="mmxp", bufs=2) as mmxp,
                tc.tile_pool(name="mmy", bufs=2) as mmy,
                tc.tile_pool(name="psum1", bufs=4, space="PSUM") as pp1,
                tc.tile_pool(name="misc", bufs=1) as misc,
            ):
                ones = misc.tile([1, 128], bf16)
                brow32 = misc.tile([1, OCOLS], f32)
                brow = misc.tile([1, OCOLS], bf16)
                wT = wts.tile([128, 32, OCOLS], bf16)   # [i%128, i//128, o]
                if do_m:
                    nc.vector.memset(ones[:], 1.0)
                    nc.sync.dma_start(brow32[:], bias_h[:, :])
                    nc.vector.tensor_copy(brow[:], brow32[:])

                with (
                    tc.tile_pool(name="xin", bufs=2) as xin_pool,
                    tc.tile_pool(name="zpool", bufs=2) as zpool,
                    tc.tile_pool(name="epool", bufs=1) as epool,
                    tc.tile_pool(name="aux", bufs=2) as aux_pool,
                    tc.tile_pool(name="scale", bufs=2) as sc_pool,
                    tc.tile_pool(name="scl", bufs=10) as scl_pool,
                    tc.tile_pool(name="work", bufs=2) as work,
                    tc.tile_pool(name="qout", bufs=2) as qout_pool,
                    tc.tile_pool(name="qconst", bufs=1) as qconst,
                ):
                    biasCA = qconst.tile([128, 1], f32)
                    if do_q:
                        nc.vector.memset(biasCA[:], CA)

                    def quant_chunk(src, dst, r0):
                        """Quantize src[r0:r0+128, :] -> dst rows (bf16)."""
                        zfull = zpool.tile([128, 64, 64], f32, tag="z")
                        scs = []
                        xhalves = []
                        for h in range(2):
                            xh = xin_pool.tile([128, 32, 64], f32, tag="x")
                            nc.sync.dma_start(
                                xh[:], src[r0:r0 + 128,
                                           2048 * h:2048 * (h + 1)].rearrange(
                                    "r (g e) -> r g e", e=64))
                            xhalves.append(xh)
                        for j in range(8):       # scales + z, 512 cols each
                            xv = xhalves[j // 4][:, 8 * (j % 4):8 * (j % 4) + 8, :]
                            ax = aux_pool.tile([128, 8, 64], f32, tag="ax")
                            nc.vector.tensor_scalar(
                                out=ax[:].bitcast(u32), in0=xv.bitcast(u32),
                                scalar1=0x7FFFFFFF, scalar2=None,
                                op0=Alu.bitwise_and)
                            m8 = sc_pool.tile([128, 8, 8], f32, tag="m8")
                            for b in range(8):
                                nc.vector.max(out=m8[:, b, :], in_=ax[:, b, :])
                            dd = sc_pool.tile([128, 8], f32, tag="dd")
                            nc.vector.tensor_tensor(
                                out=dd[:], in0=m8[:, :, 0], in1=m8[:, :, 1],
                                op=Alu.subtract)
                            sc = scl_pool.tile([128, 8], f32, tag="sc")
                            nc.vector.scalar_tensor_tensor(
                                out=sc[:], in0=dd[:], scalar=QFRAC,
                                in1=m8[:, :, 1], op0=Alu.mult, op1=Alu.add)
                            nc.vector.tensor_scalar(
                                out=sc[:], in0=sc[:], scalar1=1e-8,
                                scalar2=None, op0=Alu.max)
                            rcpC = sc_pool.tile([128, 8], f32, tag="rcpC")
                            nc.vector.reciprocal(out=rcpC[:], in_=sc[:])
                            nc.vector.tensor_scalar(
                                out=rcpC[:], in0=rcpC[:], scalar1=C_,
                                scalar2=None, op0=Alu.mult)
                            nc.gpsimd.tensor_tensor(
                                out=zfull[:, 8 * j:8 * j + 8, :], in0=xv,
                                in1=bcast64(rcpC[:]), op=Alu.mult)
                            scs.append(sc)
                        zf2 = zfull[:].rearrange("p g e -> p (g e)")
                        nc.vector.tensor_scalar(
                            out=zf2, in0=zf2, scalar1=C_, scalar2=-C_,
                            op0=Alu.min, op1=Alu.max)
                        efull = epool.tile([128, 4096], f32, tag="e")
                        nc.scalar.activation(out=efull[:], in_=zf2,
                                             func=Act.Erf, scale=ERFS)
                        for j in range(8):       # warp + snap + value, 512 cols
                            sl = slice(512 * j, 512 * (j + 1))
                            zj = zfull[:].rearrange("p g e -> p (g e)")[:, sl]
                            ej = efull[:, sl]
                            z2 = work.tile([128, 512], f32, tag="T3")
                            nc.scalar.activation(out=z2[:], in_=zj,
                                                 func=Act.Square)
                            ab = work.tile([128, 512], f32, tag="T4")
                            nc.scalar.activation(out=ab[:], in_=ej,
                                                 func=Act.Abs, scale=B_)
                            tt = work.tile([128, 512], f32, tag="T5")
                            nc.vector.scalar_tensor_tensor(
                                out=tt[:], in0=ej, scalar=A_, in1=ab[:],
                                op0=Alu.mult, op1=Alu.subtract)
                            E2 = work.tile([128, 512], f32, tag="T6")
                            nc.scalar.activation(out=E2[:], in_=z2[:],
                                                 func=Act.Exp, scale=0.5)
                            w1 = work.tile([128, 512], f32, tag="T7")
                            nc.scalar.activation(out=w1[:], in_=z2[:],
                                                 func=Act.Identity,
                                                 bias=biasCA[:], scale=CB)
                            E2z = work.tile([128, 512], f32, tag="T3")
                            nc.gpsimd.tensor_tensor(
                                out=E2z[:], in0=E2[:], in1=zj, op=Alu.mult)
                            w2 = work.tile([128, 512], f32, tag="T4")
                            nc.vector.tensor_tensor(
                                out=w2[:], in0=E2z[:], in1=w1[:], op=Alu.mult)
                            azcc = work.tile([128, 512], f32, tag="T7")
                            nc.scalar.activation(out=azcc[:], in_=E2z[:],
                                                 func=Act.Abs, scale=CC)
                            tc1 = work.tile([128, 512], f32, tag="T3")
                            nc.vector.scalar_tensor_tensor(
                                out=tc1[:], in0=w2[:], scalar=-1.0, in1=tt[:],
                                op0=Alu.mult, op1=Alu.add)
                            nc.vector.scalar_tensor_tensor(
                                out=tc1[:], in0=azcc[:], scalar=-1.0,
                                in1=tc1[:], op0=Alu.mult, op1=Alu.add)
                            kk = work.tile([128, 512], f32, tag="T7")
                            nc.vector.tensor_scalar(
                                out=kk[:], in0=tc1[:], scalar1=MAGIC,
                                scalar2=MAGIC, op0=Alu.add, op1=Alu.subtract)
                            nc.vector.tensor_scalar(
                                out=kk[:], in0=kk[:], scalar1=-16.0,
                                scalar2=15.0, op0=Alu.max, op1=Alu.min)
                            du = work.tile([128, 512], bf16, tag="T8")
                            nc.gpsimd.tensor_tensor(
                                out=du[:], in0=tt[:], in1=kk[:],
                                op=Alu.subtract)
                            sNb = work.tile([128, 512], bf16, tag="T9")
                            nc.vector.scalar_tensor_tensor(
                                out=sNb[:], in0=du[:], scalar=KAPPA,
                                in1=E2[:], op0=Alu.mult, op1=Alu.mult)
                            zb = work.tile([128, 512], bf16, tag="T10")
                            nc.scalar.activation(out=zb[:], in_=zj,
                                                 func=Act.Copy, scale=INVC)
                            v = work.tile([128, 512], bf16, tag="T11")
                            nc.vector.tensor_tensor(
                                out=v[:], in0=zb[:], in1=sNb[:],
                                op=Alu.subtract)
                            sN2 = work.tile([128, 512], bf16, tag="T7")
                            nc.scalar.activation(out=sN2[:], in_=sNb[:],
                                                 func=Act.Square)
                            h1 = work.tile([128, 512], bf16, tag="T3")
                            nc.gpsimd.tensor_tensor(
                                out=h1[:], in0=sN2[:], in1=zb[:], op=Alu.mult)
                            nc.vector.scalar_tensor_tensor(
                                out=v[:], in0=h1[:], scalar=HC2, in1=v[:],
                                op0=Alu.mult, op1=Alu.add)
                            dq = qout_pool.tile([128, 8, 64], bf16, tag="dq")
                            nc.gpsimd.tensor_tensor(
                                out=dq[:],
                                in0=v[:].rearrange("p (g e) -> p g e", e=64),
                                in1=bcast64(scs[j][:]), op=Alu.mult)
                            nc.scalar.dma_start(
                                dst[r0:r0 + 128, sl].rearrange(
                                    "r (g e) -> r g e", e=64), dq[:])

                    def load_wT_quarter(qw_half, s, half):
                        """Transpose-load [256 rows, 4096] into wT columns."""
                        for oc in range(2):
                            for k in range(32):
                                nc.sync.dma_start_transpose(
                                    wT[:, k,
                                       s * 512 + half * 256 + oc * 128:
                                       s * 512 + half * 256 + (oc + 1) * 128],
                                    qw_half[s, oc * 128:(oc + 1) * 128,
                                            k * 128:(k + 1) * 128])

                    def mm_chunk(j):
                        """y rows for gathered chunk j: 4 cp panels x 1024 o."""
                        for cp in range(4):
                            xp = mmxp.tile([128, 32, 128], bf16, tag="xp")
                            for k in range(32):
                                nc.sync.dma_start_transpose(
                                    xp[:, k, :],
                                    qx_all[j, cp, :, k * 128:(k + 1) * 128])
                            for g in range(2):
                                ps = pp1.tile([128, 512], f32, tag="ps")
                                for k in range(32):
                                    nc.tensor.matmul(
                                        ps[:], lhsT=xp[:, k, :],
                                        rhs=wT[:, k, g * 512:(g + 1) * 512],
                                        start=(k == 0), stop=False)
                                nc.tensor.matmul(
                                    ps[:], lhsT=ones[:],
                                    rhs=brow[:, g * 512:(g + 1) * 512],
                                    start=False, stop=True)
                                yb = mmy.tile([128, 512], f32, tag="yb")
                                nc.scalar.copy(yb[:], ps[:])
                                rbase = cp * 1024 + j * 128
                                nc.scalar.dma_start(
                                    y_sh[rbase:rbase + 128,
                                         g * 512:(g + 1) * 512], yb[:])

                    # ---- W phase: quant 4 chunks, AG halves early ----
                    if do_q:
                        for ch in range(NWCH):
                            quant_chunk(w_sh, qw_own, ch * 128)
                            if ch == 1:
                                nc.gpsimd.collective_compute(
                                    "AllGather", Alu.bypass,
                                    replica_groups=WG,
                                    ins=[qw_own[0:256, :].opt()],
                                    outs=[qw_allA.opt()])
                        nc.gpsimd.collective_compute(
                            "AllGather", Alu.bypass,
                            replica_groups=WG,
                            ins=[qw_own[256:512, :].opt()],
                            outs=[qw_allB.opt()])

                    # ---- X pipeline: quant j -> AG j -> mm j-1 ----
                    for j in range(NXCH):
                        if do_q:
                            quant_chunk(x_sh, qx_own, j * 128)
                        if do_m and j == 0:
                            for s in range(2):
                                load_wT_quarter(qw_allA, s, 0)
                        if do_m and j == 1:
                            for s in range(2):
                                load_wT_quarter(qw_allB, s, 1)
                        if do_q:
                            nc.gpsimd.collective_compute(
                                "AllGather", Alu.bypass,
                                replica_groups=XG,
                                ins=[qx_own[j * 128:(j + 1) * 128, :].opt()],
                                outs=[qx_all[j].opt()])
                        if do_m and j >= 1:
                            mm_chunk(j - 1)
                    if do_m:
                        mm_chunk(NXCH - 1)
    nc.compile()
    return nc


def kernel(input, weight, bias):
    from concourse.bass_utils import run_bass_kernel_spmd

    if "nc" not in _cache:
        _cache["nc"] = _build_nc(
            repeat=int(os.environ.get("KERNEL_REPEAT", "1")),
            phase=os.environ.get("KERNEL_PHASE", "all"))
    nc = _cache["nc"]

    x2 = np.ascontiguousarray(
        np.asarray(input, dtype=np.float32).reshape(RTOT, D_IN))
    w = np.asarray(weight, dtype=np.float32)
    b = np.asarray(bias, dtype=np.float32)

    in_maps = []
    for c in range(NCORES):
        r, cc = c // 4, c % 4
        xs = np.ascontiguousarray(x2[r * 4096 + cc * 1024:
                                     r * 4096 + (cc + 1) * 1024])
        ws = np.ascontiguousarray(w[cc * 1024 + r * 512:
                                    cc * 1024 + (r + 1) * 512])
        bh = np.ascontiguousarray(b[cc * 1024:(cc + 1) * 1024]).reshape(1, OCOLS)
        in_maps.append({"x_sh": xs, "w_sh": ws, "bias_h": bh})

    res = run_bass_kernel_spmd(nc, in_maps, core_ids=list(range(NCORES)))
    _cache["exec_time_ns"] = res.exec_time_ns

    y = np.empty((RTOT, D_OUT), dtype=np.float32)
    for c in range(NCORES):
        r, cc = c // 4, c % 4
        y[r * 4096:(r + 1) * 4096, cc * 1024:(cc + 1) * 1024] = \
            res.results[c]["y_sh"]
    return y.reshape(B_SZ, S_SZ, D_OUT)


# revision 6
# speedup vs baseline: 2.1103x; 1.2978x over previous
# NF5 block-quantized linear (AXSLinearV2) on 8 Trainium2 cores — v4.
#
# v4 vs v3: NO DMA-xbar transposes (measured ~1.7us per 128x128 tile — they
# dominated v2/v3 mm phase). Instead, quantized tiles are transposed on the
# PE array (is_transpose matmul vs identity, ~0.15us/tile) BEFORE the
# AllGathers, so only own data is transposed (384 tiles total) and every
# DMA in the kernel is contiguous.
#   - 2x4 grid: row-groups of 4 cores share x (4-rank chunk AllGathers of
#     transposed chunks); column pairs share w (two pair AllGathers).
#   - wT resident in SBUF [128, 32k, 1024o] bf16 (8 MB); single-pass mm.
#   - pipeline: quant+transpose chunk j overlaps mm chunk j-1.

import os
import sys

for _p in ("/opt/trn_rl_repo", "/root/.axon_site/_ro/trn_rl_repo"):
    if os.path.isdir(_p) and _p not in sys.path:
        sys.path.insert(0, _p)

import numpy as np

C_ = 2.1538746940614564        # ndtri(1 - 1/64)
A_ = 16.0                      # warp: t = A*e + B*|e|  (k-16 domain)
B_ = -0.516129032258065
CA = 0.010089541448848835      # boundary-correction coefficients
CB = -0.00019778768532179383
CC = 0.00028827144861916877
KAPPA = 0.036405892028398036   # D1*sqrt(2*pi)/C
INVC = 0.46427956220347655
HC2 = 2.3195880606018394       # C^2/2 (Halley term, for h1 = sN^2 * (z/C))
ERFS = 0.7071067811865475
QFRAC = 0.9369999999999976     # quantile(0.999) lerp fraction for n=64
MAGIC = 12582912.0             # 1.5*2^23 round-to-nearest trick

NCORES = 8
B_SZ, S_SZ, D_IN, D_OUT = 4, 2048, 4096, 4096
RTOT = B_SZ * S_SZ
XR = RTOT // NCORES            # 1024 x-rows quantized per core
WR = D_OUT // NCORES           # 512 w-rows quantized per core
NXCH = XR // 128               # 8 x chunks
NWCH = WR // 128               # 4 w chunks
OCOLS = 1024                   # output cols per core (col-group of 4 cores)
YROWS = 4096                   # output rows per core (row-group rows)

_cache = {}


def _build_nc(repeat=1, phase="all"):
    import concourse.bass as bass
    import concourse.bacc as bacc
    import concourse.tile as tile
    import concourse.masks as masks
    from concourse import mybir

    f32 = mybir.dt.float32
    bf16 = mybir.dt.bfloat16
    u32 = mybir.dt.uint32
    Alu = mybir.AluOpType
    Act = mybir.ActivationFunctionType

    do_q = phase in ("all", "quant")
    do_m = phase in ("all", "mm")

    def bcast64(ap2d):
        return bass.AP(tensor=ap2d.tensor, offset=ap2d.offset,
                       ap=[ap2d.ap[0], ap2d.ap[1], [0, 64]])

    nc = bacc.Bacc("TRN2", target_bir_lowering=False, debug=False,
                   num_devices=NCORES)
    x_sh = nc.dram_tensor("x_sh", [XR, D_IN], f32, kind="ExternalInput")
    w_sh = nc.dram_tensor("w_sh", [WR, D_IN], f32, kind="ExternalInput")
    bias_h = nc.dram_tensor("bias_h", [1, OCOLS], f32, kind="ExternalInput")
    y_sh = nc.dram_tensor("y_sh", [YROWS, OCOLS], f32, kind="ExternalOutput")

    XG = [[0, 1, 2, 3], [4, 5, 6, 7]]          # row-groups (share x)
    WG = [[c, c + 4] for c in range(4)]        # col pairs (share w)

    with tile.TileContext(nc) as tc:
     for _rep in range(repeat):
        with tc.tile_pool(name="dram", bufs=1, space="DRAM") as dram:
            # transposed layouts: [i%128 (partition), i//128 (k), rows/cols]
            qxT_own = dram.tile([NXCH, 128, 32, 128], bf16)
            qxT_all = dram.tile([NXCH, 4, 128, 32, 128], bf16)
            qwT_ownA = dram.tile([128, 32, 256], bf16)
            qwT_ownB = dram.tile([128, 32, 256], bf16)
            qwT_allA = dram.tile([2, 128, 32, 256], bf16)
            qwT_allB = dram.tile([2, 128, 32, 256], bf16)

            with (
                tc.tile_pool(name="wts", bufs=1) as wts,
                tc.tile_pool(name="mmxp", bufs=2) as mmxp,
                tc.tile_pool(name="mmy", bufs=2) as mmy,
                tc.tile_pool(name="psum1", bufs=4, space="PSUM") as pp1,
                tc.tile_pool(name="tpsum", bufs=2, space="PSUM") as tpp,
                tc.tile_pool(name="xstage", bufs=2) as xstage_pool,
                tc.tile_pool(name="wstage", bufs=1) as wstage_pool,
                tc.tile_pool(name="misc", bufs=1) as misc,
            ):
                ones = misc.tile([1, 128], bf16)
                ident = misc.tile([128, 128], bf16)
                brow32 = misc.tile([1, OCOLS], f32)
                brow = misc.tile([1, OCOLS], bf16)
                wT = wts.tile([128, 32, OCOLS], bf16)   # [i%128, i//128, o]
                wstage = wstage_pool.tile([128, 32, 256], bf16)
                masks.make_identity(nc, ident[:])
                if do_m:
                    nc.vector.memset(ones[:], 1.0)
                    nc.sync.dma_start(brow32[:], bias_h[:, :])
                    nc.vector.tensor_copy(brow[:], brow32[:])

                with (
                    tc.tile_pool(name="xin", bufs=2) as xin_pool,
                    tc.tile_pool(name="zpool", bufs=1) as zpool,
                    tc.tile_pool(name="epool", bufs=1) as epool,
                    tc.tile_pool(name="aux", bufs=2) as aux_pool,
                    tc.tile_pool(name="scale", bufs=2) as sc_pool,
                    tc.tile_pool(name="scl", bufs=10) as scl_pool,
                    tc.tile_pool(name="work", bufs=2) as work,
                    tc.tile_pool(name="qout", bufs=2) as qout_pool,
                    tc.tile_pool(name="qconst", bufs=1) as qconst,
                ):
                    biasCA = qconst.tile([128, 1], f32)
                    if do_q:
                        nc.vector.memset(biasCA[:], CA)

                    def quant_chunk(src, r0, sink):
                        """Quantize src[r0:r0+128, :]; sink(j, dq2d) consumes
                        each [128, 512] bf16 result tile."""
                        zfull = zpool.tile([128, 64, 64], f32, tag="z")
                        scs = []
                        xhalves = []
                        for h in range(2):
                            xh = xin_pool.tile([128, 32, 64], f32, tag="x")
                            nc.sync.dma_start(
                                xh[:], src[r0:r0 + 128,
                                           2048 * h:2048 * (h + 1)].rearrange(
                                    "r (g e) -> r g e", e=64))
                            xhalves.append(xh)
                        for j in range(8):       # scales + z, 512 cols each
                            xv = xhalves[j // 4][:, 8 * (j % 4):8 * (j % 4) + 8, :]
                            ax = aux_pool.tile([128, 8, 64], f32, tag="ax")
                            nc.vector.tensor_scalar(
                                out=ax[:].bitcast(u32), in0=xv.bitcast(u32),
                                scalar1=0x7FFFFFFF, scalar2=None,
                                op0=Alu.bitwise_and)
                            m8 = sc_pool.tile([128, 8, 8], f32, tag="m8")
                            for b in range(8):
                                nc.vector.max(out=m8[:, b, :], in_=ax[:, b, :])
                            dd = sc_pool.tile([128, 8], f32, tag="dd")
                            nc.vector.tensor_tensor(
                                out=dd[:], in0=m8[:, :, 0], in1=m8[:, :, 1],
                                op=Alu.subtract)
                            sc = scl_pool.tile([128, 8], f32, tag="sc")
                            nc.vector.scalar_tensor_tensor(
                                out=sc[:], in0=dd[:], scalar=QFRAC,
                                in1=m8[:, :, 1], op0=Alu.mult, op1=Alu.add)
                            nc.vector.tensor_scalar(
                                out=sc[:], in0=sc[:], scalar1=1e-8,
                                scalar2=None, op0=Alu.max)
                            rcpC = sc_pool.tile([128, 8], f32, tag="rcpC")
                            nc.vector.reciprocal(out=rcpC[:], in_=sc[:])
                            nc.vector.tensor_scalar(
                                out=rcpC[:], in0=rcpC[:], scalar1=C_,
                                scalar2=None, op0=Alu.mult)
                            nc.gpsimd.tensor_tensor(
                                out=zfull[:, 8 * j:8 * j + 8, :], in0=xv,
                                in1=bcast64(rcpC[:]), op=Alu.mult)
                            scs.append(sc)
                        zf2 = zfull[:].rearrange("p g e -> p (g e)")
                        nc.vector.tensor_scalar(
                            out=zf2, in0=zf2, scalar1=C_, scalar2=-C_,
                            op0=Alu.min, op1=Alu.max)
                        efull = epool.tile([128, 4096], f32, tag="e")
                        nc.scalar.activation(out=efull[:], in_=zf2,
                                             func=Act.Erf, scale=ERFS)
                        for j in range(8):       # warp + snap + value, 512 cols
                            sl = slice(512 * j, 512 * (j + 1))
                            zj = zfull[:].rearrange("p g e -> p (g e)")[:, sl]
                            ej = efull[:, sl]
                            z2 = work.tile([128, 512], f32, tag="T3")
                            nc.scalar.activation(out=z2[:], in_=zj,
                                                 func=Act.Square)
                            ab = work.tile([128, 512], f32, tag="T4")
                            nc.scalar.activation(out=ab[:], in_=ej,
                                                 func=Act.Abs, scale=B_)
                            tt = work.tile([128, 512], f32, tag="T5")
                            nc.vector.scalar_tensor_tensor(
                                out=tt[:], in0=ej, scalar=A_, in1=ab[:],
                                op0=Alu.mult, op1=Alu.subtract)
                            E2 = work.tile([128, 512], f32, tag="T6")
                            nc.scalar.activation(out=E2[:], in_=z2[:],
                                                 func=Act.Exp, scale=0.5)
                            w1 = work.tile([128, 512], f32, tag="T7")
                            nc.scalar.activation(out=w1[:], in_=z2[:],
                                                 func=Act.Identity,
                                                 bias=biasCA[:], scale=CB)
                            E2z = work.tile([128, 512], f32, tag="T3")
                            nc.gpsimd.tensor_tensor(
                                out=E2z[:], in0=E2[:], in1=zj, op=Alu.mult)
                            w2 = work.tile([128, 512], f32, tag="T4")
                            nc.vector.tensor_tensor(
                                out=w2[:], in0=E2z[:], in1=w1[:], op=Alu.mult)
                            azcc = work.tile([128, 512], f32, tag="T7")
                            nc.scalar.activation(out=azcc[:], in_=E2z[:],
                                                 func=Act.Abs, scale=CC)
                            tc1 = work.tile([128, 512], f32, tag="T3")
                            nc.vector.scalar_tensor_tensor(
                                out=tc1[:], in0=w2[:], scalar=-1.0, in1=tt[:],
                                op0=Alu.mult, op1=Alu.add)
                            nc.vector.scalar_tensor_tensor(
                                out=tc1[:], in0=azcc[:], scalar=-1.0,
                                in1=tc1[:], op0=Alu.mult, op1=Alu.add)
                            kk = work.tile([128, 512], f32, tag="T7")
                            nc.vector.tensor_scalar(
                                out=kk[:], in0=tc1[:], scalar1=MAGIC,
                                scalar2=MAGIC, op0=Alu.add, op1=Alu.subtract)
                            nc.vector.tensor_scalar(
                                out=kk[:], in0=kk[:], scalar1=-16.0,
                                scalar2=15.0, op0=Alu.max, op1=Alu.min)
                            du = work.tile([128, 512], bf16, tag="T8")
                            nc.gpsimd.tensor_tensor(
                                out=du[:], in0=tt[:], in1=kk[:],
                                op=Alu.subtract)
                            sNb = work.tile([128, 512], bf16, tag="T9")
                            nc.vector.scalar_tensor_tensor(
                                out=sNb[:], in0=du[:], scalar=KAPPA,
                                in1=E2[:], op0=Alu.mult, op1=Alu.mult)
                            zb = work.tile([128, 512], bf16, tag="T10")
                            nc.scalar.activation(out=zb[:], in_=zj,
                                                 func=Act.Copy, scale=INVC)
                            v = work.tile([128, 512], bf16, tag="T11")
                            nc.vector.tensor_tensor(
                                out=v[:], in0=zb[:], in1=sNb[:],
                                op=Alu.subtract)
                            sN2 = work.tile([128, 512], bf16, tag="T7")
                            nc.scalar.activation(out=sN2[:], in_=sNb[:],
                                                 func=Act.Square)
                            h1 = work.tile([128, 512], bf16, tag="T3")
                            nc.gpsimd.tensor_tensor(
                                out=h1[:], in0=sN2[:], in1=zb[:], op=Alu.mult)
                            nc.vector.scalar_tensor_tensor(
                                out=v[:], in0=h1[:], scalar=HC2, in1=v[:],
                                op0=Alu.mult, op1=Alu.add)
                            dq = qout_pool.tile([128, 8, 64], bf16, tag="dq")
                            nc.gpsimd.tensor_tensor(
                                out=dq[:],
                                in0=v[:].rearrange("p (g e) -> p g e", e=64),
                                in1=bcast64(scs[j][:]), op=Alu.mult)
                            sink(j, dq[:].rearrange("p g e -> p (g e)"))

                    def transpose_to(j, dq2d, stage_dst):
                        """PE-transpose 4 [128,128] tiles of dq into stage."""
                        tp = tpp.tile([128, 1024], bf16, tag="tp")
                        for t in range(4):
                            nc.tensor.matmul(
                                tp[:, t * 128:(t + 1) * 128],
                                lhsT=dq2d[:, t * 128:(t + 1) * 128],
                                rhs=ident[:], is_transpose=True)
                        nc.scalar.copy(stage_dst, tp[:, 0:512])

                    def mm_chunk(j):
                        """y rows for gathered chunk j: 4 cp panels x 1024 o."""
                        for cp in range(4):
                            xp = mmxp.tile([128, 32, 128], bf16, tag="xp")
                            nc.sync.dma_start(xp[:], qxT_all[j, cp])
                            for g in range(2):
                                ps = pp1.tile([128, 512], f32, tag="ps")
                                for k in range(32):
                                    nc.tensor.matmul(
                                        ps[:], lhsT=xp[:, k, :],
                                        rhs=wT[:, k, g * 512:(g + 1) * 512],
                                        start=(k == 0), stop=False)
                                nc.tensor.matmul(
                                    ps[:], lhsT=ones[:],
                                    rhs=brow[:, g * 512:(g + 1) * 512],
                                    start=False, stop=True)
                                yb = mmy.tile([128, 512], f32, tag="yb")
                                nc.scalar.copy(yb[:], ps[:])
                                rbase = cp * 1024 + j * 128
                                nc.scalar.dma_start(
                                    y_sh[rbase:rbase + 128,
                                         g * 512:(g + 1) * 512], yb[:])

                    def w_phase():
                        for ch in range(NWCH):
                            quant_chunk(
                                w_sh, ch * 128,
                                lambda jj, dq2d, _c=ch: transpose_to(
                                    jj, dq2d,
                                    wstage[:, 4 * jj:4 * jj + 4,
                                           (_c % 2) * 128:
                                           (_c % 2 + 1) * 128]))
                            if ch == 1:
                                nc.sync.dma_start(qwT_ownA[:], wstage[:])
                                nc.gpsimd.collective_compute(
                                    "AllGather", Alu.bypass,
                                    replica_groups=WG,
                                    ins=[qwT_ownA.opt()],
                                    outs=[qwT_allA.opt()])
                        nc.sync.dma_start(qwT_ownB[:], wstage[:])
                        nc.gpsimd.collective_compute(
                            "AllGather", Alu.bypass,
                            replica_groups=WG,
                            ins=[qwT_ownB.opt()],
                            outs=[qwT_allB.opt()])

                    def x_iter(j):
                        if do_q:
                            xstage = xstage_pool.tile([128, 32, 128], bf16,
                                                      tag="xs")
                            quant_chunk(
                                x_sh, j * 128,
                                lambda jj, dq2d, _s=xstage: transpose_to(
                                    jj, dq2d, _s[:, 4 * jj:4 * jj + 4, :]))
                            nc.sync.dma_start(qxT_own[j], xstage[:])
                        if do_m and j == 0:
                            for s in range(2):
                                nc.sync.dma_start(
                                    wT[:, :, s * 512:s * 512 + 256],
                                    qwT_allA[s])
                        if do_m and j == 1:
                            for s in range(2):
                                nc.sync.dma_start(
                                    wT[:, :, s * 512 + 256:(s + 1) * 512],
                                    qwT_allB[s])
                        if do_q:
                            nc.gpsimd.collective_compute(
                                "AllGather", Alu.bypass,
                                replica_groups=XG,
                                ins=[qxT_own[j].opt()],
                                outs=[qxT_all[j].opt()])
                        if do_m and j >= 1:
                            mm_chunk(j - 1)

                    # ---- W phase, then X pipeline: quant j ; AG j ; mm j-1 --
                    if do_q:
                        w_phase()
                    for j in range(NXCH):
                        x_iter(j)
                    if do_m:
                        mm_chunk(NXCH - 1)
    nc.compile()
    return nc


def kernel(input, weight, bias):
    from concourse.bass_utils import run_bass_kernel_spmd

    if "nc" not in _cache:
        _cache["nc"] = _build_nc(
            repeat=int(os.environ.get("KERNEL_REPEAT", "1")),
            phase=os.environ.get("KERNEL_PHASE", "all"))
    nc = _cache["nc"]

    x2 = np.ascontiguousarray(
        np.asarray(input, dtype=np.float32).reshape(RTOT, D_IN))
    w = np.asarray(weight, dtype=np.float32)
    b = np.asarray(bias, dtype=np.float32)

    in_maps = []
    for c in range(NCORES):
        r, cc = c // 4, c % 4
        xs = np.ascontiguousarray(x2[r * 4096 + cc * 1024:
                                     r * 4096 + (cc + 1) * 1024])
        ws = np.ascontiguousarray(w[cc * 1024 + r * 512:
                                    cc * 1024 + (r + 1) * 512])
        bh = np.ascontiguousarray(b[cc * 1024:(cc + 1) * 1024]).reshape(1, OCOLS)
        in_maps.append({"x_sh": xs, "w_sh": ws, "bias_h": bh})

    res = run_bass_kernel_spmd(nc, in_maps, core_ids=list(range(NCORES)))
    _cache["exec_time_ns"] = res.exec_time_ns

    y = np.empty((RTOT, D_OUT), dtype=np.float32)
    for c in range(NCORES):
        r, cc = c // 4, c % 4
        y[r * 4096:(r + 1) * 4096, cc * 1024:(cc + 1) * 1024] = \
            res.results[c]["y_sh"]
    return y.reshape(B_SZ, S_SZ, D_OUT)
